# revision 22
# baseline (speedup 1.0000x reference)
"""Dropless MoE (top-2 of 8 experts) for Trainium2, 8 NeuronCores.

Sharding: expert x F-half. Core c runs two sequential jobs:
  job A = (expert c,        F[:2048])
  job B = (expert (c+1)%8,  F[2048:])
so each job's weights (8.4 MB w1-half + 8.4 MB w2-half, float32r) are
fully SBUF-resident and every weight byte is DMA'd exactly once per core
(33.6 MB). The host computes the router (validated to match jax.lax.top_k
exactly, incl. tie order), gathers each expert's tokens into a padded
[H, CAP] float32 buffer, and scatter-adds the two F-half partial outputs
per expert. All matmuls run in float32r: ~1.5e-4 matmul rel err at full
1 cyc/row PE rate (free dims kept >= 256 and 16-aligned). End-to-end
error vs the fp32 reference: ~1.9e-4 of output scale. Cost-model
(TimelineSim) estimate ~254 us/core; measured ~310-420 us on the shared
axon terminal depending on core count.

Device per job: for each token tile (>=256 wide):
  up:   hT[f,t]  = silu( sum_d w1[d,f] * xt[d,t] )   (PSUM accum over 8 d-chunks)
  down: y[h,t]   = cw[t] * sum_f w2[f,h] * hT[f,t]   (PSUM accum over 16 f-chunks)
"""

import numpy as np

H = 1024
F = 4096
E = 8
TOP_K = 2
FH = F // 2  # F-half per job
N_CORES = 8

_cache = {}


def _build_device_kernel(cap, tts, sim_safe=False, with_b2=True, repeat=1, dtype_tag="f32r"):
    import concourse.bass as bass  # noqa: F401
    import concourse.mybir as mybir
    import concourse.tile as tile
    from concourse import bacc

    F32 = mybir.dt.float32
    F32R = {"f32r": mybir.dt.float32r,
            "bf16": mybir.dt.bfloat16,
            "f32": mybir.dt.float32}[dtype_tag]
    SILU = mybir.ActivationFunctionType.Silu

    nc = bacc.Bacc("TRN2", target_bir_lowering=False, debug=False,
                   num_devices=N_CORES)

    params = {}
    for j in ("a", "b"):
        params[f"x{j}"] = nc.declare_dram_parameter(f"x{j}", [H, cap], F32R, isOutput=False)
        params[f"w1{j}"] = nc.declare_dram_parameter(f"w1{j}", [H, FH], F32R, isOutput=False)
        params[f"w2{j}"] = nc.declare_dram_parameter(f"w2{j}", [FH, H], F32R, isOutput=False)
        params[f"cw{j}"] = nc.declare_dram_parameter(f"cw{j}", [128, cap], F32, isOutput=False)
        params[f"b1{j}"] = nc.declare_dram_parameter(f"b1{j}", [FH], F32, isOutput=False)
        params[f"y{j}"] = nc.declare_dram_parameter(
            f"y{j}", [H, cap], F32, isOutput=True)
    params["b2a"] = nc.declare_dram_parameter("b2a", [H], F32, isOutput=False)

    NDO = H // 128    # 8 contraction chunks for up-proj
    NFO = FH // 128   # 16 f-chunks per half
    NHO = H // 128    # 8 output h-chunks

    with tile.TileContext(nc) as tc:
        with (
            tc.tile_pool(name="wpool", bufs=1) as wpool,
            tc.tile_pool(name="xpool", bufs=2) as xpool,
            tc.tile_pool(name="hpool", bufs=1) as hpool,
            tc.tile_pool(name="cpool", bufs=1) as cpool,
            tc.tile_pool(name="ypool", bufs=3) as ypool,
            tc.tile_pool(name="ps", bufs=8, space="PSUM") as psp,
        ):
            from contextlib import nullcontext
            rep_ctx = tc.For_i(0, repeat, 1) if repeat > 1 else nullcontext()
            with rep_ctx:
              for j in ("a", "b"):
                  xr = params[f"x{j}"].rearrange("(do p) t -> p do t", p=128)
                  w1r = params[f"w1{j}"].rearrange("(do p) f -> p do f", p=128)
                  w2r = params[f"w2{j}"].rearrange("(fo p) h -> p fo h", p=128)
                  yr = params[f"y{j}"].rearrange("(ho p) t -> p ho t", p=128)

                  ttmax = max(tts)
                  # DMA issue order == consumption order: tile-0 activations
                  # and the first w1 column block gate the first matmul chain;
                  # b1 gates the first silu eviction (psum recycling); cw/b2
                  # gate the first down-proj eviction; w2 gates down-proj.
                  x_tiles = [None] * len(tts)
                  x_full = xpool.tile([128, NDO, ttmax], F32R, tag="x", name="x_full")
                  x_tiles[0] = x_full[:, :, :tts[0]]
                  for do in range(NDO):
                      nc.sync.dma_start(x_tiles[0][:, do], xr[:, do, 0:tts[0]])
                  # w1 as 4 quarter tiles / w2 as 2 half tiles: the next job's
                  # loads begin as soon as this job's last read of each piece
                  # retires, hiding the weight swap behind remaining compute
                  w1_q = []
                  for q in range(4):
                      w1_t = wpool.tile([128, NDO, 512], F32R, tag=f"w1q{q}",
                                        name=f"w1_t{q}")
                      w1_q.append(w1_t)
                  for do in range(NDO):
                      nc.sync.dma_start(w1_q[0][:, do], w1r[:, do, 0:512])
                  b1_sb = cpool.tile([128, NFO], F32, tag="b1")
                  nc.sync.dma_start(
                      b1_sb[:], params[f"b1{j}"].rearrange("(fo p) -> p fo", p=128))
                  if j == "a" and with_b2:
                      b2_sb = cpool.tile([128, NHO], F32, tag="b2")
                      nc.sync.dma_start(
                          b2_sb[:], params["b2a"].rearrange("(ho p) -> p ho", p=128))
                  cw_sb = cpool.tile([128, cap], F32, tag="cw")
                  nc.sync.dma_start(cw_sb[:], params[f"cw{j}"][:])
                  for fq in range(512, FH, 512):
                      for do in range(NDO):
                          nc.sync.dma_start(
                              w1_q[fq // 512][:, do], w1r[:, do, fq:fq + 512])
                  w2_h = []
                  for hh in range(2):
                      w2_t = wpool.tile([128, NFO, 512], F32R, tag=f"w2h{hh}",
                                        name=f"w2_t{hh}")
                      w2_h.append(w2_t)
                  for hq in range(0, H, 512):
                      for fo in range(NFO):
                          nc.sync.dma_start(
                              w2_h[hq // 512][:, fo], w2r[:, fo, hq:hq + 512])
                  t0 = 0
                  for ti, tt in enumerate(tts):
                      sl = slice(t0, t0 + tt)
                      if x_tiles[ti] is None:
                          x_full = xpool.tile([128, NDO, ttmax], F32R, tag="x",
                                              name="x_full")
                          x_tiles[ti] = x_full[:, :, :tt]
                          for do in range(NDO):
                              nc.sync.dma_start(x_tiles[ti][:, do], xr[:, do, sl])
                      x_sb = x_tiles[ti]
                      h_full = hpool.tile([128, NFO, ttmax], F32R, tag="h", name="h_full")
                      hT = h_full[:, :, :tt]
                      for fo in range(NFO):
                          ps = psp.tile([128, 512], mybir.dt.float32, tag="ps", name="ps_u")
                          for do in range(NDO):
                              nc.tensor.matmul(
                                  ps[:, :tt],
                                  w1_q[fo // 4][:, do,
                                                (fo % 4) * 128:(fo % 4 + 1) * 128],
                                  x_sb[:, do],
                                  start=(do == 0),
                                  stop=(do == NDO - 1),
                              )
                          if sim_safe:
                              # CoreSim has no Silu table: z*sigmoid(z)
                              sg = ypool.tile([128, 512], F32, tag="sg")
                              nc.scalar.activation(
                                  sg[:, :tt], ps[:, :tt],
                                  mybir.ActivationFunctionType.Sigmoid,
                                  bias=b1_sb[:, fo:fo + 1])
                              zb = ypool.tile([128, 512], F32, tag="zb")
                              nc.scalar.activation(
                                  zb[:, :tt], ps[:, :tt],
                                  mybir.ActivationFunctionType.Identity,
                                  bias=b1_sb[:, fo:fo + 1])
                              nc.vector.tensor_mul(
                                  hT[:, fo], zb[:, :tt], sg[:, :tt])
                          else:
                              nc.scalar.activation(
                                  hT[:, fo], ps[:, :tt], SILU,
                                  bias=b1_sb[:, fo:fo + 1])
                      for ho in range(NHO):
                          ps = psp.tile([128, 512], mybir.dt.float32, tag="ps", name="ps_d")
                          for fo in range(NFO):
                              nc.tensor.matmul(
                                  ps[:, :tt],
                                  w2_h[ho // 4][:, fo,
                                                (ho % 4) * 128:(ho % 4 + 1) * 128],
                                  hT[:, fo],
                                  start=(fo == 0),
                                  stop=(fo == NFO - 1),
                              )
                          y_sb = ypool.tile([128, 512], F32, tag="y")
                          if j == "a" and with_b2:
                              # y = (psum + b2[h]) * cw  (b2 added once per expert)
                              nc.scalar.activation(
                                  y_sb[:, :tt], ps[:, :tt],
                                  mybir.ActivationFunctionType.Identity,
                                  bias=b2_sb[:, ho:ho + 1])
                              nc.vector.tensor_mul(
                                  y_sb[:, :tt], y_sb[:, :tt], cw_sb[:, sl])
                          else:
                              nc.vector.tensor_mul(
                                  y_sb[:, :tt], ps[:, :tt], cw_sb[:, sl])
                          nc.sync.dma_start(yr[:, ho, sl], y_sb[:, :tt])
                      t0 += tt
    nc.compile()
    return nc


def _token_tiles(cap):
    """Split cap (multiple of 16) into balanced 16-aligned tiles each <=512
    and >=256 when possible (float32r matmuls need 16-aligned free dims and
    run at full rate only for free dim >=256)."""
    assert cap % 16 == 0
    units = cap // 16
    n = max(1, -(-cap // 512))
    base, extra = divmod(units, n)
    tts = [16 * (base + (1 if i < extra else 0)) for i in range(n)]
    assert sum(tts) == cap, (cap, tts)
    return tts


def kernel(x, gate_w, w1, b1, w2, b2):
    from concourse.bass_utils import run_bass_kernel_spmd

    x = np.asarray(x, dtype=np.float32)
    gate_w = np.asarray(gate_w, dtype=np.float32)
    w1 = np.asarray(w1, dtype=np.float32)
    b1 = np.asarray(b1, dtype=np.float32)
    w2 = np.asarray(w2, dtype=np.float32)
    b2 = np.asarray(b2, dtype=np.float32)

    B, S, _H = x.shape
    T = B * S
    y = x.reshape(T, _H)

    # ---- Router on host (fp32; matches jax.lax.top_k incl. tie order) ----
    logits = y @ gate_w.T                                   # [T, E]
    m = logits.max(axis=-1, keepdims=True)
    ex = np.exp(logits - m, dtype=np.float32)
    probs = ex / ex.sum(axis=-1, keepdims=True)
    idx = np.argsort(-probs, axis=-1, kind="stable")[:, :TOP_K]   # [T, k]
    ew = np.take_along_axis(probs, idx, axis=-1)                  # [T, k]

    # z_loss (mirror jax logsumexp in fp32)
    log_z = np.log(ex.sum(axis=-1, dtype=np.float32)) + m[:, 0]
    z_loss = np.float32((log_z.astype(np.float32) ** 2).sum() / T)

    # load-balancing aux loss (reference uses num_classes = k)
    k = TOP_K
    oh = (idx[..., None] == np.arange(k)).astype(np.int32)        # [T, k, k]
    mask = oh.max(axis=-2)
    tokens_per_expert = mask.astype(np.float32).mean(axis=-2)
    router_prob_per_expert = ew.astype(np.float32).mean(axis=-2)
    aux_loss = np.float32(
        (tokens_per_expert * router_prob_per_expert).mean() * (k ** 2))

    rw = ew / ew.sum(axis=-1, keepdims=True)                      # [T, k]

    # ---- Per-expert gather ----
    tok = []
    cwl = []
    for e in range(E):
        sel = (idx[:, 0] == e) | (idx[:, 1] == e)
        te = np.nonzero(sel)[0]
        w_slot = np.where(idx[te, 0] == e, rw[te, 0], rw[te, 1])
        tok.append(te)
        cwl.append(w_slot.astype(np.float32))
    counts = [len(t) for t in tok]
    cap = max(256, ((max(counts) + 15) // 16) * 16)
    tts = _token_tiles(cap)

    xt = np.zeros((E, H, cap), np.float32)
    cw = np.zeros((E, 128, cap), np.float32)
    for e in range(E):
        xt[e, :, :counts[e]] = y[tok[e]].T
        cw[e, :, :counts[e]] = cwl[e][None, :]

    with_b2 = bool(np.any(b2))
    key = (cap, tuple(tts), with_b2)
    if key not in _cache:
        _cache[key] = _build_device_kernel(cap, tts, with_b2=with_b2)
    nc = _cache[key]

    in_maps = []
    for c in range(N_CORES):
        ea, eb = c, (c + 1) % E
        in_maps.append({
            "xa": xt[ea], "xb": xt[eb],
            "w1a": np.ascontiguousarray(w1[ea, :, :FH]),
            "w1b": np.ascontiguousarray(w1[eb, :, FH:]),
            "w2a": np.ascontiguousarray(w2[ea, :FH, :]),
            "w2b": np.ascontiguousarray(w2[eb, FH:, :]),
            "cwa": cw[ea], "cwb": cw[eb],
            "b1a": np.ascontiguousarray(b1[ea, :FH]),
            "b1b": np.ascontiguousarray(b1[eb, FH:]),
            "b2a": b2[ea],
        })

    res = run_bass_kernel_spmd(nc, in_maps, core_ids=list(range(N_CORES)))

    out = np.zeros((T, H), np.float32)
    for e in range(E):
        ya = res.results[e]["ya"]                # (expert e, F[:2048])
        yb = res.results[(e - 1) % E]["yb"]      # (expert e, F[2048:])
        out[tok[e]] += (ya + yb).T[:counts[e]]

    return out.reshape(B, S, H), z_loss, aux_loss



# revision 24
# speedup vs baseline: 1.0201x; 1.0201x over previous
"""Dropless MoE (top-2 of 8 experts) for Trainium2, 8 NeuronCores.

Sharding: expert x F-half. Core c runs two sequential jobs:
  job A = (expert c,        F[:2048])
  job B = (expert (c+1)%8,  F[2048:])
so each job's weights (8.4 MB w1-half + 8.4 MB w2-half, float32r) are
fully SBUF-resident and every weight byte is DMA'd exactly once per core
(33.6 MB). The host computes the router (validated to match jax.lax.top_k
exactly, incl. tie order), gathers each expert's tokens into a padded
[H, CAP] float32 buffer, and scatter-adds the two F-half partial outputs
per expert. All matmuls run in float32r: ~1.5e-4 matmul rel err at full
1 cyc/row PE rate (free dims kept >= 256 and 16-aligned). End-to-end
error vs the fp32 reference: ~1.9e-4 of output scale. Cost-model
(TimelineSim) estimate ~254 us/core; measured ~310-420 us on the shared
axon terminal depending on core count.

Device per job: for each token tile (>=256 wide):
  up:   hT[f,t]  = silu( sum_d w1[d,f] * xt[d,t] )   (PSUM accum over 8 d-chunks)
  down: y[h,t]   = cw[t] * sum_f w2[f,h] * hT[f,t]   (PSUM accum over 16 f-chunks)
"""

import numpy as np

H = 1024
F = 4096
E = 8
TOP_K = 2
FH = F // 2  # F-half per job
N_CORES = 8

_cache = {}


def _build_device_kernel(caps, ttss, sim_safe=False, with_b2=True, repeat=1, dtype_tag="f32r"):
    import concourse.bass as bass  # noqa: F401
    import concourse.mybir as mybir
    import concourse.tile as tile
    from concourse import bacc

    F32 = mybir.dt.float32
    F32R = {"f32r": mybir.dt.float32r,
            "bf16": mybir.dt.bfloat16,
            "f32": mybir.dt.float32}[dtype_tag]
    SILU = mybir.ActivationFunctionType.Silu

    nc = bacc.Bacc("TRN2", target_bir_lowering=False, debug=False,
                   num_devices=N_CORES)

    params = {}
    for j in ("a", "b"):
        cap_j = caps[j]
        params[f"x{j}"] = nc.declare_dram_parameter(f"x{j}", [H, cap_j], F32R, isOutput=False)
        params[f"w1{j}"] = nc.declare_dram_parameter(f"w1{j}", [H, FH], F32R, isOutput=False)
        params[f"w2{j}"] = nc.declare_dram_parameter(f"w2{j}", [FH, H], F32R, isOutput=False)
        params[f"cw{j}"] = nc.declare_dram_parameter(f"cw{j}", [128, cap_j], F32, isOutput=False)
        params[f"b1{j}"] = nc.declare_dram_parameter(f"b1{j}", [FH], F32, isOutput=False)
        params[f"y{j}"] = nc.declare_dram_parameter(
            f"y{j}", [H, cap_j], F32, isOutput=True)
    params["b2a"] = nc.declare_dram_parameter("b2a", [H], F32, isOutput=False)
    ttmax_all = max(max(t) for t in ttss.values())

    NDO = H // 128    # 8 contraction chunks for up-proj
    NFO = FH // 128   # 16 f-chunks per half
    NHO = H // 128    # 8 output h-chunks

    with tile.TileContext(nc) as tc:
        with (
            tc.tile_pool(name="wpool", bufs=1) as wpool,
            tc.tile_pool(name="xpool", bufs=2) as xpool,
            tc.tile_pool(name="hpool", bufs=1) as hpool,
            tc.tile_pool(name="cpool", bufs=1) as cpool,
            tc.tile_pool(name="ypool", bufs=3) as ypool,
            tc.tile_pool(name="ps", bufs=8, space="PSUM") as psp,
        ):
            from contextlib import nullcontext
            rep_ctx = tc.For_i(0, repeat, 1) if repeat > 1 else nullcontext()
            with rep_ctx:
              for j in ("a", "b"):
                  cap = caps[j]
                  tts = ttss[j]
                  xr = params[f"x{j}"].rearrange("(do p) t -> p do t", p=128)
                  w1r = params[f"w1{j}"].rearrange("(do p) f -> p do f", p=128)
                  w2r = params[f"w2{j}"].rearrange("(fo p) h -> p fo h", p=128)
                  yr = params[f"y{j}"].rearrange("(ho p) t -> p ho t", p=128)

                  ttmax = ttmax_all
                  # DMA issue order == consumption order: tile-0 activations
                  # and the first w1 column block gate the first matmul chain;
                  # b1 gates the first silu eviction (psum recycling); cw/b2
                  # gate the first down-proj eviction; w2 gates down-proj.
                  x_tiles = [None] * len(tts)
                  x_full = xpool.tile([128, NDO, ttmax], F32R, tag="x", name="x_full")
                  x_tiles[0] = x_full[:, :, :tts[0]]
                  for do in range(NDO):
                      nc.sync.dma_start(x_tiles[0][:, do], xr[:, do, 0:tts[0]])
                  # w1 as 4 quarter tiles / w2 as 2 half tiles: the next job's
                  # loads begin as soon as this job's last read of each piece
                  # retires, hiding the weight swap behind remaining compute
                  w1_q = []
                  for q in range(4):
                      w1_t = wpool.tile([128, NDO, 512], F32R, tag=f"w1q{q}",
                                        name=f"w1_t{q}")
                      w1_q.append(w1_t)
                  for do in range(NDO):
                      nc.sync.dma_start(w1_q[0][:, do], w1r[:, do, 0:512])
                  b1_sb = cpool.tile([128, NFO], F32, tag="b1")
                  nc.sync.dma_start(
                      b1_sb[:], params[f"b1{j}"].rearrange("(fo p) -> p fo", p=128))
                  if j == "a" and with_b2:
                      b2_sb = cpool.tile([128, NHO], F32, tag="b2")
                      nc.sync.dma_start(
                          b2_sb[:], params["b2a"].rearrange("(ho p) -> p ho", p=128))
                  cw_sb = cpool.tile([128, cap], F32, tag="cw")
                  nc.sync.dma_start(cw_sb[:], params[f"cw{j}"][:])
                  for fq in range(512, FH, 512):
                      for do in range(NDO):
                          nc.sync.dma_start(
                              w1_q[fq // 512][:, do], w1r[:, do, fq:fq + 512])
                  w2_h = []
                  for hh in range(2):
                      w2_t = wpool.tile([128, NFO, 512], F32R, tag=f"w2h{hh}",
                                        name=f"w2_t{hh}")
                      w2_h.append(w2_t)
                  for hq in range(0, H, 512):
                      for fo in range(NFO):
                          nc.sync.dma_start(
                              w2_h[hq // 512][:, fo], w2r[:, fo, hq:hq + 512])
                  t0 = 0
                  for ti, tt in enumerate(tts):
                      sl = slice(t0, t0 + tt)
                      if x_tiles[ti] is None:
                          x_full = xpool.tile([128, NDO, ttmax], F32R, tag="x",
                                              name="x_full")
                          x_tiles[ti] = x_full[:, :, :tt]
                          for do in range(NDO):
                              nc.sync.dma_start(x_tiles[ti][:, do], xr[:, do, sl])
                      x_sb = x_tiles[ti]
                      h_full = hpool.tile([128, NFO, ttmax], F32R, tag="h", name="h_full")
                      hT = h_full[:, :, :tt]
                      for fo in range(NFO):
                          ps = psp.tile([128, 512], mybir.dt.float32, tag="ps", name="ps_u")
                          for do in range(NDO):
                              nc.tensor.matmul(
                                  ps[:, :tt],
                                  w1_q[fo // 4][:, do,
                                                (fo % 4) * 128:(fo % 4 + 1) * 128],
                                  x_sb[:, do],
                                  start=(do == 0),
                                  stop=(do == NDO - 1),
                              )
                          if sim_safe:
                              # CoreSim has no Silu table: z*sigmoid(z)
                              sg = ypool.tile([128, 512], F32, tag="sg")
                              nc.scalar.activation(
                                  sg[:, :tt], ps[:, :tt],
                                  mybir.ActivationFunctionType.Sigmoid,
                                  bias=b1_sb[:, fo:fo + 1])
                              zb = ypool.tile([128, 512], F32, tag="zb")
                              nc.scalar.activation(
                                  zb[:, :tt], ps[:, :tt],
                                  mybir.ActivationFunctionType.Identity,
                                  bias=b1_sb[:, fo:fo + 1])
                              nc.vector.tensor_mul(
                                  hT[:, fo], zb[:, :tt], sg[:, :tt])
                          else:
                              nc.scalar.activation(
                                  hT[:, fo], ps[:, :tt], SILU,
                                  bias=b1_sb[:, fo:fo + 1])
                      for ho in range(NHO):
                          ps = psp.tile([128, 512], mybir.dt.float32, tag="ps", name="ps_d")
                          for fo in range(NFO):
                              nc.tensor.matmul(
                                  ps[:, :tt],
                                  w2_h[ho // 4][:, fo,
                                                (ho % 4) * 128:(ho % 4 + 1) * 128],
                                  hT[:, fo],
                                  start=(fo == 0),
                                  stop=(fo == NFO - 1),
                              )
                          y_sb = ypool.tile([128, 512], F32, tag="y")
                          if j == "a" and with_b2:
                              # y = (psum + b2[h]) * cw  (b2 added once per expert)
                              nc.scalar.activation(
                                  y_sb[:, :tt], ps[:, :tt],
                                  mybir.ActivationFunctionType.Identity,
                                  bias=b2_sb[:, ho:ho + 1])
                              nc.vector.tensor_mul(
                                  y_sb[:, :tt], y_sb[:, :tt], cw_sb[:, sl])
                          else:
                              nc.vector.tensor_mul(
                                  y_sb[:, :tt], ps[:, :tt], cw_sb[:, sl])
                          nc.sync.dma_start(yr[:, ho, sl], y_sb[:, :tt])
                      t0 += tt
    nc.compile()
    return nc


def _token_tiles(cap):
    """Split cap (multiple of 16) into balanced 16-aligned tiles each <=512
    and >=256 when possible (float32r matmuls need 16-aligned free dims and
    run at full rate only for free dim >=256)."""
    assert cap % 16 == 0
    units = cap // 16
    n = max(1, -(-cap // 512))
    base, extra = divmod(units, n)
    tts = [16 * (base + (1 if i < extra else 0)) for i in range(n)]
    assert sum(tts) == cap, (cap, tts)
    return tts


def kernel(x, gate_w, w1, b1, w2, b2):
    from concourse.bass_utils import run_bass_kernel_spmd

    x = np.asarray(x, dtype=np.float32)
    gate_w = np.asarray(gate_w, dtype=np.float32)
    w1 = np.asarray(w1, dtype=np.float32)
    b1 = np.asarray(b1, dtype=np.float32)
    w2 = np.asarray(w2, dtype=np.float32)
    b2 = np.asarray(b2, dtype=np.float32)

    B, S, _H = x.shape
    T = B * S
    y = x.reshape(T, _H)

    # ---- Router on host (fp32; matches jax.lax.top_k incl. tie order) ----
    logits = y @ gate_w.T                                   # [T, E]
    m = logits.max(axis=-1, keepdims=True)
    ex = np.exp(logits - m, dtype=np.float32)
    probs = ex / ex.sum(axis=-1, keepdims=True)
    idx = np.argsort(-probs, axis=-1, kind="stable")[:, :TOP_K]   # [T, k]
    ew = np.take_along_axis(probs, idx, axis=-1)                  # [T, k]

    # z_loss (mirror jax logsumexp in fp32)
    log_z = np.log(ex.sum(axis=-1, dtype=np.float32)) + m[:, 0]
    z_loss = np.float32((log_z.astype(np.float32) ** 2).sum() / T)

    # load-balancing aux loss (reference uses num_classes = k)
    k = TOP_K
    oh = (idx[..., None] == np.arange(k)).astype(np.int32)        # [T, k, k]
    mask = oh.max(axis=-2)
    tokens_per_expert = mask.astype(np.float32).mean(axis=-2)
    router_prob_per_expert = ew.astype(np.float32).mean(axis=-2)
    aux_loss = np.float32(
        (tokens_per_expert * router_prob_per_expert).mean() * (k ** 2))

    rw = ew / ew.sum(axis=-1, keepdims=True)                      # [T, k]

    # ---- Per-expert gather ----
    tok = []
    cwl = []
    for e in range(E):
        sel = (idx[:, 0] == e) | (idx[:, 1] == e)
        te = np.nonzero(sel)[0]
        w_slot = np.where(idx[te, 0] == e, rw[te, 0], rw[te, 1])
        tok.append(te)
        cwl.append(w_slot.astype(np.float32))
    counts = [len(t) for t in tok]
    with_b2 = bool(np.any(b2))

    def align(c):
        return max(256, ((c + 15) // 16) * 16)

    # Job assignment: 16 half-jobs (expert x F-half) onto 8 cores x 2 slots.
    # With b2 == 0 the slots are structurally identical, so slot A takes both
    # halves of the 4 largest experts and slot B the 4 smallest, letting
    # slot B pad to a smaller capacity (~2% less compute + better balance).
    # Non-zero b2 is added structurally in slot A, which forces half-0 into
    # slot A for every expert (the symmetric layout).
    if with_b2:
        a_jobs = [(c, 0) for c in range(N_CORES)]
        b_jobs = [((c + 1) % E, 1) for c in range(N_CORES)]
    else:
        order = sorted(range(E), key=lambda e: -counts[e])
        a_jobs = [(e, h) for e in order[:E // 2] for h in (0, 1)]
        b_jobs = [(e, h) for e in order[E // 2:] for h in (0, 1)]
    cap_a = align(max(counts[e] for e, _ in a_jobs))
    cap_b = align(max(counts[e] for e, _ in b_jobs))
    caps = {"a": cap_a, "b": cap_b}
    ttss = {"a": _token_tiles(cap_a), "b": _token_tiles(cap_b)}

    key = (cap_a, tuple(ttss["a"]), cap_b, tuple(ttss["b"]), with_b2)
    if key not in _cache:
        _cache[key] = _build_device_kernel(caps, ttss, with_b2=with_b2)
    nc = _cache[key]

    xts = {}
    cws = {}
    for j, jobs in (("a", a_jobs), ("b", b_jobs)):
        cap_j = caps[j]
        for e in {e for e, _ in jobs}:
            xe = np.zeros((H, cap_j), np.float32)
            xe[:, :counts[e]] = y[tok[e]].T
            xts[(j, e)] = xe
            ce = np.zeros((128, cap_j), np.float32)
            ce[:, :counts[e]] = cwl[e][None, :]
            cws[(j, e)] = ce

    def job_inputs(j, e, h):
        fsl = slice(0, FH) if h == 0 else slice(FH, F)
        return {
            f"x{j}": xts[(j, e)],
            f"w1{j}": np.ascontiguousarray(w1[e, :, fsl]),
            f"w2{j}": np.ascontiguousarray(w2[e, fsl, :]),
            f"cw{j}": cws[(j, e)],
            f"b1{j}": np.ascontiguousarray(b1[e, fsl]),
        }

    in_maps = []
    for c in range(N_CORES):
        m = {}
        m.update(job_inputs("a", *a_jobs[c]))
        m.update(job_inputs("b", *b_jobs[c]))
        m["b2a"] = b2[a_jobs[c][0]]
        in_maps.append(m)

    res = run_bass_kernel_spmd(nc, in_maps, core_ids=list(range(N_CORES)))

    out = np.zeros((T, H), np.float32)
    partial = {}
    for c in range(N_CORES):
        partial[a_jobs[c]] = res.results[c]["ya"]
        partial[b_jobs[c]] = res.results[c]["yb"]
    for e in range(E):
        cnt = counts[e]
        y_e = partial[(e, 0)][:, :cnt] + partial[(e, 1)][:, :cnt]
        out[tok[e]] += y_e.T

    return out.reshape(B, S, H), z_loss, aux_loss

